# revision 4
# baseline (speedup 1.0000x reference)
"""TRN2 Bass kernel for GCNConv-diag: out = A @ (input * diag(W)).

Strategy (8 NeuronCores, SPMD):
  - Shard A row-wise: core i owns rows [i*1024, (i+1)*1024).
  - Replicate the feature matrix `input` (matmul moving operand) and W.
  - W commutes into the features: fold W into x on the host
    (x = input * W), so the device matmul is plain A @ x.
  - Mean-subtraction + fp8 DoubleRow: A = 0.5*J + B with B = A - 0.5
    (A is uniform[0,1), so B is symmetric in [-0.5, 0.5)).  Then
      A @ x = 0.5 * colsum(x) (rank-1, exact, host-computed)  +  B @ x.
    B @ x runs on the PE in float8e4 (e4m3): 2 fp8 weights per PE cell
    -> one matmul instruction contracts 2 k-tiles (K=256).  The exact
    rank-1 mean term is added on the host after the gather.
  - Weights use DoubleRowSwInterleave: the host pre-interleaves each
    stationary pair column-reversed ([A127,B127,A126,...,B0]).  Measured
    identical speed to hardware DoubleRow (no FWL either way); plain
    DoubleRow fallback via _SWI = False (host layout switches with it).
  - The feature matrix x (fp8) is loaded into SBUF once in the prologue
    and stays resident across the repeat loop: the steady-state work per
    iteration is streaming the A shard (8.4 MB fp8) + writing out
    (1 MB fp16).  Output converts f32 psum -> fp16 on the DVE drain;
    the host upconverts and adds the rank-1 mean term.

Per-core work: out[1024,512] = B_shard[1024,8192] @ x[8192,512]:
8 m-tiles x 32 DoubleRow matmuls ([128,2,128]^T x [128,2,512] -> psum
[128,512], 32-deep accumulation).

Roofline: 4.295 GMAC/core at the fp8-DoubleRow peak of 32768 MACs/cycle
@2.4GHz = 54.6us/core; the LDWEIGHTS stream (65536 weight-columns
@1.2GHz = 54.6us) is exactly balanced with it.  Measured steady-state
(repeat-slope) is 53.6-57.1us = ~98% of peak: both PE ports saturated.
DMA is 9.4MB/rep (~26us) and fully hidden; cutting bytes further or
flipping the matmul orientation cannot help, and sub-fp8 dtypes or
one-level Strassen would blow the 2e-2 error budget (fp8 quantization
already costs 1.78e-2).
"""

import numpy as np
import ml_dtypes

import concourse.bass as bass
import concourse.tile as tile
from concourse import bacc, mybir
from concourse.bass_utils import run_bass_kernel_spmd

N = 8192  # graph nodes (A is [N, N])
D = 512  # feature dim
NCORES = 8
RPC = N // NCORES  # 1024 rows of A / output per core
MT = RPC // 128  # 8 output m-tiles per core
KT = N // 128  # 64 contraction k-tiles

NP8 = KT // 2  # DoubleRow pair count (32)
ACH = 2  # A panel chunks per m-tile
SC8 = KT // ACH  # k-subtiles per A chunk (32)
PPC = SC8 // 2  # pairs per A chunk (16)
XCH = 8  # x chunk count
SX = KT // XCH  # k-subtiles per x chunk (8)

_SWI = True  # software-interleaved weights (FWL-eligible LDWEIGHTS)

_F32 = mybir.dt.float32
_F16 = mybir.dt.float16
_FP8 = mybir.dt.float8e4
_NP_FP8 = ml_dtypes.float8_e4m3  # IEEE-ish e4m3 (max 240) == TRN FP8_EXP4
_DR = (
    mybir.MatmulPerfMode.DoubleRowSwInterleave
    if _SWI
    else mybir.MatmulPerfMode.DoubleRow
)

_compiled = None
_last_in_maps = None


def _build(repeats=1):
    nc = bacc.Bacc("TRN2", target_bir_lowering=False, debug=False, num_devices=NCORES)
    # a8[m, p, s*128+c] = B[m*128+c, s*128+p] as e4m3  (s = k-subtile;
    # under _SWI each pair of s-subtiles is interleaved column-reversed)
    a8 = nc.dram_tensor("a8", [MT, 128, KT * 128], _FP8, kind="ExternalInput").ap()
    # x8[p, s*512+d] = x[s*128+p, d] as e4m3
    x8 = nc.dram_tensor("x8", [128, KT * D], _FP8, kind="ExternalInput").ap()
    out = nc.dram_tensor("out", [RPC, D], _F16, kind="ExternalOutput").ap()

    with tile.TileContext(nc) as tc:
        with (
            tc.tile_pool(name="xp", bufs=1) as xp,
            tc.tile_pool(name="apool", bufs=8) as apool,
            tc.tile_pool(name="op", bufs=8) as op,
            tc.tile_pool(name="ps", bufs=8, space="PSUM") as ps,
        ):
            # Prologue: x chunks load once and stay resident (32KB/partition).
            x_tiles = []
            for c in range(XCH):
                xt = xp.tile([128, SX, D], _FP8, tag=f"x{c}")
                nc.sync.dma_start(out=xt[:], in_=x8[:, c * SX * D : (c + 1) * SX * D])
                x_tiles.append(xt)

            def load_a(m):
                ts = []
                for c in range(ACH):
                    a_t = apool.tile([128, SC8, 128], _FP8, tag="a8")
                    nc.sync.dma_start(
                        out=a_t[:],
                        in_=a8[m, :, c * SC8 * 128 : (c + 1) * SC8 * 128],
                    )
                    ts.append(a_t)
                return ts

            # Flatten (rep, m) so A prefetch pipelines across rep bounds.
            total = repeats * MT
            a_pending = {0: load_a(0)}
            if total > 1:
                a_pending[1] = load_a(1 % MT)

            for t in range(total):
                m = t % MT
                a_tiles = a_pending.pop(t)
                psum = ps.tile([128, D], _F32)
                for i in range(NP8):
                    lhsT = a_tiles[i // PPC][:, (i % PPC) * 2 : (i % PPC) * 2 + 2, :]
                    s = 2 * i
                    rhs = x_tiles[s // SX][:, (s % SX) : (s % SX) + 2, :]
                    nc.tensor.matmul(
                        psum[:],
                        lhsT,
                        rhs,
                        start=(i == 0),
                        stop=(i == NP8 - 1),
                        perf_mode=_DR,
                    )
                if t + 2 < total:
                    a_pending[t + 2] = load_a((t + 2) % MT)
                o_t = op.tile([128, D], _F16)
                nc.vector.tensor_copy(o_t[:], psum[:])
                nc.sync.dma_start(out=out[m * 128 : (m + 1) * 128, :], in_=o_t[:])

    nc.compile()
    return nc


def _get_compiled():
    global _compiled
    if _compiled is None:
        _compiled = _build()
    return _compiled


def _host_prep(input, A, W):
    """Quantize + rearrange full inputs into per-core in_maps."""
    x = input * W[None, :]  # fold diag(W) into the features
    # x8[p, s*512+d] = x[s*128+p, d]
    xr = x.reshape(KT, 128, D).transpose(1, 0, 2).reshape(128, KT * D)
    x8 = np.ascontiguousarray(xr).astype(_NP_FP8)

    in_maps = []
    for i in range(NCORES):
        b_shard = A[i * RPC : (i + 1) * RPC] - np.float32(0.5)
        # atm[m, p, s*128+c] = b_shard[m*128+c, s*128+p]
        atm = (
            b_shard.reshape(MT, 128, KT, 128)
            .transpose(0, 3, 2, 1)
            .reshape(MT, 128, KT, 128)
        )
        if _SWI:
            # per stationary pair (s=2t, 2t+1): [A127,B127,A126,...,B0]
            st = atm.reshape(MT, 128, KT // 2, 2, 128)
            a8 = np.ascontiguousarray(
                st[:, :, :, :, ::-1].transpose(0, 1, 2, 4, 3).reshape(MT, 128, KT * 128)
            ).astype(_NP_FP8)
        else:
            a8 = np.ascontiguousarray(atm.reshape(MT, 128, KT * 128)).astype(_NP_FP8)
        in_maps.append({"a8": a8, "x8": x8})
    return in_maps


def kernel(input, A, W):
    input = np.ascontiguousarray(np.asarray(input, dtype=np.float32))
    A = np.ascontiguousarray(np.asarray(A, dtype=np.float32))
    W = np.ascontiguousarray(np.asarray(W, dtype=np.float32))

    nc = _get_compiled()
    in_maps = _host_prep(input, A, W)

    global _last_in_maps
    _last_in_maps = in_maps

    x = input * W[None, :]
    mean_row = (0.5 * x.sum(axis=0, dtype=np.float64)).astype(np.float32)

    res = run_bass_kernel_spmd(nc, in_maps, list(range(NCORES)))
    out = np.concatenate(
        [np.asarray(res.results[i]["out"]) for i in range(NCORES)], axis=0
    ).astype(np.float32)
    return out + mean_row[None, :]


# revision 5
# speedup vs baseline: 1.1551x; 1.1551x over previous
"""TRN2 Bass kernel for GCNConv-diag: out = A @ (input * diag(W)).

Strategy (8 NeuronCores, SPMD):
  - Shard A row-wise: core i owns rows [i*1024, (i+1)*1024).
  - Replicate the feature matrix `input` (matmul moving operand) and W.
  - W commutes into the features: fold W into x on the host
    (x = input * W), so the device matmul is plain A @ x.
  - Mean-subtraction + fp8 DoubleRow: A = 0.5*J + B with B = A - 0.5
    (A is uniform[0,1), so B is symmetric in [-0.5, 0.5)).  Then
      A @ x = 0.5 * colsum(x) (rank-1, exact, host-computed)  +  B @ x.
    B @ x runs on the PE in float8e4 (e4m3): 2 fp8 weights per PE cell
    -> one matmul instruction contracts 2 k-tiles (K=256).  The exact
    rank-1 mean term is added on the host after the gather.
  - Weights use DoubleRowSwInterleave: the host pre-interleaves each
    stationary pair column-reversed ([A127,B127,A126,...,B0]).  Measured
    identical speed to hardware DoubleRow (no FWL either way); plain
    DoubleRow fallback via _SWI = False (host layout switches with it).
  - The feature matrix x (fp8) is loaded into SBUF once in the prologue
    and stays resident across the repeat loop: the steady-state work per
    iteration is streaming the A shard (8.4 MB fp8) + writing out
    (1 MB fp16).  Output converts f32 psum -> fp16 on the DVE drain;
    the host upconverts and adds the rank-1 mean term.

Per-core work: out[1024,512] = B_shard[1024,8192] @ x[8192,512]:
8 m-tiles x 32 DoubleRow matmuls ([128,2,128]^T x [128,2,512] -> psum
[128,512], 32-deep accumulation).

Roofline: 4.295 GMAC/core at the fp8-DoubleRow peak of 32768 MACs/cycle
@2.4GHz = 54.6us/core; the LDWEIGHTS stream (65536 weight-columns
@1.2GHz = 54.6us) is exactly balanced with it.  Measured steady-state
(repeat-slope) is 53.6-57.1us = ~98% of peak: both PE ports saturated.
DMA is 9.4MB/rep (~26us) and fully hidden; cutting bytes further or
flipping the matmul orientation cannot help, and sub-fp8 dtypes or
one-level Strassen would blow the 2e-2 error budget (fp8 quantization
already costs 1.78e-2).
"""

import numpy as np
import ml_dtypes

import concourse.bass as bass
import concourse.tile as tile
from concourse import bacc, mybir
from concourse.bass_utils import run_bass_kernel_spmd

N = 8192  # graph nodes (A is [N, N])
D = 512  # feature dim
NCORES = 8
RPC = N // NCORES  # 1024 rows of A / output per core
MT = RPC // 128  # 8 output m-tiles per core
KT = N // 128  # 64 contraction k-tiles

NP8 = KT // 2  # DoubleRow pair count (32)
ACH = 2  # A panel chunks per m-tile
SC8 = KT // ACH  # k-subtiles per A chunk (32)
PPC = SC8 // 2  # pairs per A chunk (16)
XCH = 8  # x chunk count
SX = KT // XCH  # k-subtiles per x chunk (8)

_SWI = True  # software-interleaved weights (FWL-eligible LDWEIGHTS)

_F32 = mybir.dt.float32
_F16 = mybir.dt.float16
_FP8 = mybir.dt.float8e4
_NP_FP8 = ml_dtypes.float8_e4m3  # IEEE-ish e4m3 (max 240) == TRN FP8_EXP4
_DR = (
    mybir.MatmulPerfMode.DoubleRowSwInterleave
    if _SWI
    else mybir.MatmulPerfMode.DoubleRow
)

_compiled = None
_last_in_maps = None


def _build(repeats=1):
    nc = bacc.Bacc("TRN2", target_bir_lowering=False, debug=False, num_devices=NCORES)
    # a8[m, p, s*128+c] = B[m*128+c, s*128+p] as e4m3  (s = k-subtile;
    # under _SWI each pair of s-subtiles is interleaved column-reversed)
    a8 = nc.dram_tensor("a8", [MT, 128, KT * 128], _FP8, kind="ExternalInput").ap()
    # x8[p, s*512+d] = x[s*128+p, d] as e4m3
    x8 = nc.dram_tensor("x8", [128, KT * D], _FP8, kind="ExternalInput").ap()
    out = nc.dram_tensor("out", [RPC, D], _F16, kind="ExternalOutput").ap()

    with tile.TileContext(nc) as tc:
        with (
            tc.tile_pool(name="xp", bufs=1) as xp,
            tc.tile_pool(name="apool", bufs=8) as apool,
            tc.tile_pool(name="op", bufs=8) as op,
            tc.tile_pool(name="ps", bufs=8, space="PSUM") as ps,
        ):
            # Prologue: x chunks load once and stay resident (32KB/partition).
            x_tiles = []
            for c in range(XCH):
                xt = xp.tile([128, SX, D], _FP8, tag=f"x{c}")
                nc.sync.dma_start(out=xt[:], in_=x8[:, c * SX * D : (c + 1) * SX * D])
                x_tiles.append(xt)

            def load_a(m):
                ts = []
                for c in range(ACH):
                    a_t = apool.tile([128, SC8, 128], _FP8, tag="a8")
                    nc.sync.dma_start(
                        out=a_t[:],
                        in_=a8[m, :, c * SC8 * 128 : (c + 1) * SC8 * 128],
                    )
                    ts.append(a_t)
                return ts

            # Flatten (rep, m) so A prefetch pipelines across rep bounds.
            total = repeats * MT
            a_pending = {0: load_a(0)}
            if total > 1:
                a_pending[1] = load_a(1 % MT)

            for t in range(total):
                m = t % MT
                a_tiles = a_pending.pop(t)
                psum = ps.tile([128, D], _F32)
                for i in range(NP8):
                    lhsT = a_tiles[i // PPC][:, (i % PPC) * 2 : (i % PPC) * 2 + 2, :]
                    s = 2 * i
                    rhs = x_tiles[s // SX][:, (s % SX) : (s % SX) + 2, :]
                    nc.tensor.matmul(
                        psum[:],
                        lhsT,
                        rhs,
                        start=(i == 0),
                        stop=(i == NP8 - 1),
                        perf_mode=_DR,
                    )
                if t + 2 < total:
                    a_pending[t + 2] = load_a((t + 2) % MT)
                o_t = op.tile([128, D], _F16)
                nc.vector.tensor_copy(o_t[:], psum[:])
                # out-writes ride the Activation HWDGE queue so they never
                # queue behind A-prefetches on the SP queue (bank recycling
                # would otherwise wait on head-of-line A traffic)
                nc.scalar.dma_start(out=out[m * 128 : (m + 1) * 128, :], in_=o_t[:])

    nc.compile()
    return nc


def _get_compiled():
    global _compiled
    if _compiled is None:
        _compiled = _build()
    return _compiled


def _host_prep(input, A, W):
    """Quantize + rearrange full inputs into per-core in_maps."""
    x = input * W[None, :]  # fold diag(W) into the features
    # x8[p, s*512+d] = x[s*128+p, d]
    xr = x.reshape(KT, 128, D).transpose(1, 0, 2).reshape(128, KT * D)
    x8 = np.ascontiguousarray(xr).astype(_NP_FP8)

    in_maps = []
    for i in range(NCORES):
        b_shard = A[i * RPC : (i + 1) * RPC] - np.float32(0.5)
        # atm[m, p, s*128+c] = b_shard[m*128+c, s*128+p]
        atm = (
            b_shard.reshape(MT, 128, KT, 128)
            .transpose(0, 3, 2, 1)
            .reshape(MT, 128, KT, 128)
        )
        if _SWI:
            # per stationary pair (s=2t, 2t+1): [A127,B127,A126,...,B0]
            st = atm.reshape(MT, 128, KT // 2, 2, 128)
            a8 = np.ascontiguousarray(
                st[:, :, :, :, ::-1].transpose(0, 1, 2, 4, 3).reshape(MT, 128, KT * 128)
            ).astype(_NP_FP8)
        else:
            a8 = np.ascontiguousarray(atm.reshape(MT, 128, KT * 128)).astype(_NP_FP8)
        in_maps.append({"a8": a8, "x8": x8})
    return in_maps


def kernel(input, A, W):
    input = np.ascontiguousarray(np.asarray(input, dtype=np.float32))
    A = np.ascontiguousarray(np.asarray(A, dtype=np.float32))
    W = np.ascontiguousarray(np.asarray(W, dtype=np.float32))

    nc = _get_compiled()
    in_maps = _host_prep(input, A, W)

    global _last_in_maps
    _last_in_maps = in_maps

    x = input * W[None, :]
    mean_row = (0.5 * x.sum(axis=0, dtype=np.float64)).astype(np.float32)

    res = run_bass_kernel_spmd(nc, in_maps, list(range(NCORES)))
    out = np.concatenate(
        [np.asarray(res.results[i]["out"]) for i in range(NCORES)], axis=0
    ).astype(np.float32)
    return out + mean_row[None, :]
